# revision 5
# baseline (speedup 1.0000x reference)
"""Trainium2 Bass kernel for ClassWiseRegressionLoss.

reference semantics:
    idx = labels - 1                       # [N] in [0, C)
    class_pred[i] = pred[i, idx[i], :]     # [N, 2] gather
    d = class_pred - targets               # [N, 2]
    smooth_l1 = where(|d| < 1, 0.5 d^2, |d| - 0.5)
    out = mean(smooth_l1) * 2              # scalar f32

Strategy (data-parallel over N across 8 cores):
  The indirect-DMA path tops out at 128 descriptors per instruction
  (~1.5us each, 256 needed -> 383us). Instead, stream the pred shard
  through SBUF as bf16 (26 MB/core, ~75us at the measured 378 GB/s/core)
  and do the per-proposal class selection on-chip with the GPSIMD
  ap_gather extended instruction. ap_gather shares one index list per
  16-partition group, so each slot gathers 16 channels of which one is
  the targeted proposal (15/16 discarded via a constant mask; the waste
  runs on otherwise-idle lanes). Per chunk of 8192 proposals each Q7
  core gathers 1024 slots (~34 ns/slot measured -> ~139us/core total,
  the kernel's critical path, with DMA/DVE/ACT hidden underneath).

  Smooth-L1 uses the abs-free split
      sum(smooth) = 0.5*(sum(d^2) - sum(r1^2) - sum(m2^2))
  with r1 = max(d-1,0), m2 = min(d+1,0) on masked d (mask zeroes the
  15 junk channel-values per slot; smooth(0)=0). Each core returns
  [128, 3*nchunks] f32 per-partition accumulator sums; the host reduces
  in float64 and scales by 1/N. pred/targets are converted to bf16 on
  the host (pure dtype conversion; all indexing happens on-device),
  which costs ~1e-5 relative error on the loss.
"""

import functools

import numpy as np
import ml_dtypes

import concourse.bacc as bacc
import concourse.mybir as mybir
import concourse.tile as tile
from concourse.bass_utils import run_bass_kernel_spmd

N = 262144
C = 200
NCORES = 8
NLOC = N // NCORES  # 32768 proposals per core
P = 128  # SBUF partitions
M = 64  # proposals per channel per chunk
NCH = NLOC // (P * M)  # 4 chunks
NI = 16 * M  # 1024 ap_gather slots per 16-partition group per chunk
NE = M * C  # 12800 index-able pairs per channel per chunk

bf16 = mybir.dt.bfloat16
f32 = mybir.dt.float32
i16 = mybir.dt.int16


@functools.lru_cache(maxsize=1)
def _build():
    nc = bacc.Bacc(None, target_bir_lowering=False, debug=False)

    pred_t = nc.declare_dram_parameter("pred", [NLOC * C * 2], bf16, isOutput=False)
    idx_t = nc.declare_dram_parameter("idx", [NCH, P, NI // 16], i16, isOutput=False)
    trep_t = nc.declare_dram_parameter("trep", [NCH, P, 2 * NI], bf16, isOutput=False)
    mask_t = nc.declare_dram_parameter("mask", [P, 2 * NI], bf16, isOutput=False)
    out_t = nc.declare_dram_parameter("partial", [P, 3 * NCH], f32, isOutput=True)

    with tile.TileContext(nc) as tc:
        with (
            tc.tile_pool(name="io", bufs=1) as io,
            tc.tile_pool(name="src", bufs=2) as srcp,
            tc.tile_pool(name="work", bufs=2) as work,
        ):
            mask = io.tile([P, 2 * NI], bf16)
            nc.sync.dma_start(out=mask[:], in_=mask_t[:, :])
            part = io.tile([P, 3 * NCH], f32)
            # single garbage sink for the three Square outputs (only the
            # accumulator matters); WAW deps just serialize ACT, which is
            # far off the critical path
            junk = io.tile([P, 2 * NI], bf16)

            for c in range(NCH):
                src = srcp.tile([P, 2 * NE], bf16)
                nc.sync.dma_start(
                    out=src[:],
                    in_=pred_t[c * P * 2 * NE : (c + 1) * P * 2 * NE].rearrange(
                        "(p f) -> p f", p=P
                    ),
                )
                idx = srcp.tile([P, NI // 16], i16)
                nc.sync.dma_start(out=idx[:], in_=idx_t[c, :, :])
                trep = srcp.tile([P, 2 * NI], bf16)
                nc.sync.dma_start(out=trep[:], in_=trep_t[c, :, :])

                g = work.tile([P, 2 * NI], bf16)
                nc.gpsimd.ap_gather(
                    out_ap=g[:], in_ap=src[:], idxs_ap=idx[:],
                    channels=P, num_elems=NE, d=2, num_idxs=NI,
                )

                d0 = work.tile([P, 2 * NI], bf16)
                nc.vector.tensor_tensor(
                    out=d0[:], in0=g[:], in1=trep[:], op=mybir.AluOpType.subtract
                )
                dm = work.tile([P, 2 * NI], bf16)
                nc.vector.tensor_tensor(
                    out=dm[:], in0=d0[:], in1=mask[:], op=mybir.AluOpType.mult
                )
                # relu(|d|-1)^2 = r1^2 + m2^2, r1 = max(d-1,0), m2 = min(d+1,0)
                r1 = work.tile([P, 2 * NI], bf16)
                nc.vector.tensor_scalar(
                    out=r1[:], in0=dm[:], scalar1=-1.0, scalar2=0.0,
                    op0=mybir.AluOpType.add, op1=mybir.AluOpType.max,
                )
                m2 = work.tile([P, 2 * NI], bf16)
                nc.vector.tensor_scalar(
                    out=m2[:], in0=dm[:], scalar1=1.0, scalar2=0.0,
                    op0=mybir.AluOpType.add, op1=mybir.AluOpType.min,
                )
                nc.scalar.activation(
                    out=junk[:], in_=dm[:],
                    func=mybir.ActivationFunctionType.Square,
                    accum_out=part[:, 3 * c : 3 * c + 1],
                )
                nc.scalar.activation(
                    out=junk[:], in_=r1[:],
                    func=mybir.ActivationFunctionType.Square,
                    accum_out=part[:, 3 * c + 1 : 3 * c + 2],
                )
                nc.scalar.activation(
                    out=junk[:], in_=m2[:],
                    func=mybir.ActivationFunctionType.Square,
                    accum_out=part[:, 3 * c + 2 : 3 * c + 3],
                )
            nc.sync.dma_start(out=out_t[:, :], in_=part[:])

    nc.compile()
    return nc


@functools.lru_cache(maxsize=1)
def _mask_const() -> np.ndarray:
    # slot s useful on channel ch iff s % 16 == ch % 16 (both components)
    ch = np.arange(P)[:, None]
    s = np.arange(NI)[None, :]
    m = (s % 16 == ch % 16).astype(np.float32)
    return np.repeat(m, 2, axis=1).astype(ml_dtypes.bfloat16)


def _host_inputs(labels_core, targets_core):
    """Per-core idx [NCH, P, NI//16] i16 and t_rep [NCH, P, 2*NI] bf16."""
    lab = labels_core.astype(np.int64).reshape(NCH, P, M) - 1  # class in [0, C)
    u = np.arange(M)[None, None, :]
    idx = (u * C + lab).astype(np.int16)  # q for (chunk, ch, u)
    # slot s of group g targets proposal n(ch=16g+s%16, u=s//16);
    # idx element for slot s is stored at (partition s%16 of group, col s//16),
    # which is exactly idx[c, ch, u] with ch_local = s%16, u = s//16.
    tgt = targets_core.reshape(NCH, 8, 16, M, 2)  # (chunk, group, ch_local, u, t)
    # t_rep[c, ch, s, t] = targets[c, g(ch), s%16, s//16, t]  (same for all ch in group)
    tr = tgt.transpose(0, 1, 3, 2, 4)  # (chunk, group, u, ch_local, t)
    tr = tr.reshape(NCH, 8, 1, NI * 2)
    tr = np.broadcast_to(tr, (NCH, 8, 16, NI * 2)).reshape(NCH, P, 2 * NI)
    return idx, tr.astype(ml_dtypes.bfloat16)


def _run(pred, labels, targets, trace=False):
    labels = np.asarray(labels)
    targets = np.asarray(targets)
    assert pred.shape == (N, C, 2), pred.shape
    assert labels.shape == (N,), labels.shape
    assert targets.shape == (N, 2), targets.shape
    pred_bf = np.asarray(pred, dtype=np.float32).astype(ml_dtypes.bfloat16)

    nc = _build()
    mask = _mask_const()
    in_maps = []
    for c in range(NCORES):
        sl = slice(c * NLOC, (c + 1) * NLOC)
        idx, trep = _host_inputs(labels[sl], targets[sl])
        in_maps.append(
            {
                "pred": pred_bf[sl].reshape(NLOC * C * 2),
                "idx": idx,
                "trep": trep,
                "mask": mask,
            }
        )
    res = run_bass_kernel_spmd(nc, in_maps, list(range(NCORES)), trace=trace)
    total = 0.0
    for r in res.results:
        p = r["partial"].astype(np.float64).reshape(P, NCH, 3)
        # sum(smooth_l1) = 0.5*(sum(d^2) - sum(r1^2) - sum(m2^2))
        total += 0.5 * (p[:, :, 0].sum() - p[:, :, 1].sum() - p[:, :, 2].sum())
    loss = np.float32(total / N)  # = mean * 2 over 2N elements
    return loss, res


def kernel(pred, labels, targets):
    loss, _ = _run(pred, labels, targets)
    return np.asarray(loss, dtype=np.float32)


# revision 6
# speedup vs baseline: 1.0583x; 1.0583x over previous
"""Trainium2 Bass kernel for ClassWiseRegressionLoss.

reference semantics:
    idx = labels - 1                       # [N] in [0, C)
    class_pred[i] = pred[i, idx[i], :]     # [N, 2] gather
    d = class_pred - targets               # [N, 2]
    smooth_l1 = where(|d| < 1, 0.5 d^2, |d| - 0.5)
    out = mean(smooth_l1) * 2              # scalar f32

Strategy (data-parallel over N across 8 cores):
  The indirect-DMA path tops out at 128 descriptors per instruction
  (~1.5us each, 256 needed -> 383us). Instead, stream the pred shard
  through SBUF as bf16 (26 MB/core, ~75us at the measured 378 GB/s/core)
  and do the per-proposal class selection on-chip with the GPSIMD
  ap_gather extended instruction. ap_gather shares one index list per
  16-partition group, so each slot gathers 16 channels of which one is
  the targeted proposal (15/16 discarded via a constant mask; the waste
  runs on otherwise-idle lanes). Per chunk of 8192 proposals each Q7
  core gathers 1024 slots (~34 ns/slot measured -> ~139us/core total,
  the kernel's critical path, with DMA/DVE/ACT hidden underneath).

  Smooth-L1 uses the abs-free split
      sum(smooth) = 0.5*(sum(d^2) - sum(r1^2) - sum(m2^2))
  with r1 = max(d-1,0), m2 = min(d+1,0) on masked d (mask zeroes the
  15 junk channel-values per slot; smooth(0)=0). Each core returns
  [128, 3*nchunks] f32 per-partition accumulator sums; the host reduces
  in float64 and scales by 1/N. pred/targets are converted to bf16 on
  the host (pure dtype conversion; all indexing happens on-device),
  which costs ~1e-5 relative error on the loss.
"""

import functools

import numpy as np
import ml_dtypes

import concourse.bacc as bacc
import concourse.mybir as mybir
import concourse.tile as tile
from concourse.bass_utils import run_bass_kernel_spmd

N = 262144
C = 200
NCORES = 8
NLOC = N // NCORES  # 32768 proposals per core
P = 128  # SBUF partitions
M = 32  # proposals per channel per chunk
NCH = NLOC // (P * M)  # 4 chunks
NI = 16 * M  # 1024 ap_gather slots per 16-partition group per chunk
NE = M * C  # 12800 index-able pairs per channel per chunk

bf16 = mybir.dt.bfloat16
f32 = mybir.dt.float32
i16 = mybir.dt.int16


@functools.lru_cache(maxsize=1)
def _build():
    nc = bacc.Bacc(None, target_bir_lowering=False, debug=False)

    pred_t = nc.declare_dram_parameter("pred", [NLOC * C * 2], bf16, isOutput=False)
    idx_t = nc.declare_dram_parameter("idx", [NCH, P, NI // 16], i16, isOutput=False)
    trep_t = nc.declare_dram_parameter("trep", [NCH, P, 2 * NI], bf16, isOutput=False)
    mask_t = nc.declare_dram_parameter("mask", [P, 2 * NI], bf16, isOutput=False)
    out_t = nc.declare_dram_parameter("partial", [P, 3 * NCH], f32, isOutput=True)

    with tile.TileContext(nc) as tc:
        with (
            tc.tile_pool(name="io", bufs=1) as io,
            tc.tile_pool(name="src", bufs=2) as srcp,
            tc.tile_pool(name="work", bufs=2) as work,
        ):
            mask = io.tile([P, 2 * NI], bf16)
            nc.sync.dma_start(out=mask[:], in_=mask_t[:, :])
            part = io.tile([P, 3 * NCH], f32)
            # single garbage sink for the three Square outputs (only the
            # accumulator matters); WAW deps just serialize ACT, which is
            # far off the critical path
            junk = io.tile([P, 2 * NI], bf16)

            for c in range(NCH):
                src = srcp.tile([P, 2 * NE], bf16)
                nc.sync.dma_start(
                    out=src[:],
                    in_=pred_t[c * P * 2 * NE : (c + 1) * P * 2 * NE].rearrange(
                        "(p f) -> p f", p=P
                    ),
                )
                idx = srcp.tile([P, NI // 16], i16)
                nc.sync.dma_start(out=idx[:], in_=idx_t[c, :, :])
                trep = srcp.tile([P, 2 * NI], bf16)
                nc.sync.dma_start(out=trep[:], in_=trep_t[c, :, :])

                g = work.tile([P, 2 * NI], bf16)
                nc.gpsimd.ap_gather(
                    out_ap=g[:], in_ap=src[:], idxs_ap=idx[:],
                    channels=P, num_elems=NE, d=2, num_idxs=NI,
                )

                d0 = work.tile([P, 2 * NI], bf16)
                nc.vector.tensor_tensor(
                    out=d0[:], in0=g[:], in1=trep[:], op=mybir.AluOpType.subtract
                )
                dm = work.tile([P, 2 * NI], bf16)
                nc.vector.tensor_tensor(
                    out=dm[:], in0=d0[:], in1=mask[:], op=mybir.AluOpType.mult
                )
                # relu(|d|-1)^2 = r1^2 + m2^2, r1 = max(d-1,0), m2 = min(d+1,0)
                r1 = work.tile([P, 2 * NI], bf16)
                nc.vector.tensor_scalar(
                    out=r1[:], in0=dm[:], scalar1=-1.0, scalar2=0.0,
                    op0=mybir.AluOpType.add, op1=mybir.AluOpType.max,
                )
                m2 = work.tile([P, 2 * NI], bf16)
                nc.vector.tensor_scalar(
                    out=m2[:], in0=dm[:], scalar1=1.0, scalar2=0.0,
                    op0=mybir.AluOpType.add, op1=mybir.AluOpType.min,
                )
                nc.scalar.activation(
                    out=junk[:], in_=dm[:],
                    func=mybir.ActivationFunctionType.Square,
                    accum_out=part[:, 3 * c : 3 * c + 1],
                )
                nc.scalar.activation(
                    out=junk[:], in_=r1[:],
                    func=mybir.ActivationFunctionType.Square,
                    accum_out=part[:, 3 * c + 1 : 3 * c + 2],
                )
                nc.scalar.activation(
                    out=junk[:], in_=m2[:],
                    func=mybir.ActivationFunctionType.Square,
                    accum_out=part[:, 3 * c + 2 : 3 * c + 3],
                )
            nc.sync.dma_start(out=out_t[:, :], in_=part[:])

    nc.compile()
    return nc


@functools.lru_cache(maxsize=1)
def _mask_const() -> np.ndarray:
    # slot s useful on channel ch iff s % 16 == ch % 16 (both components)
    ch = np.arange(P)[:, None]
    s = np.arange(NI)[None, :]
    m = (s % 16 == ch % 16).astype(np.float32)
    return np.repeat(m, 2, axis=1).astype(ml_dtypes.bfloat16)


def _host_inputs(labels_core, targets_core):
    """Per-core idx [NCH, P, NI//16] i16 and t_rep [NCH, P, 2*NI] bf16."""
    lab = labels_core.astype(np.int64).reshape(NCH, P, M) - 1  # class in [0, C)
    u = np.arange(M)[None, None, :]
    idx = (u * C + lab).astype(np.int16)  # q for (chunk, ch, u)
    # slot s of group g targets proposal n(ch=16g+s%16, u=s//16);
    # idx element for slot s is stored at (partition s%16 of group, col s//16),
    # which is exactly idx[c, ch, u] with ch_local = s%16, u = s//16.
    tgt = targets_core.reshape(NCH, 8, 16, M, 2)  # (chunk, group, ch_local, u, t)
    # t_rep[c, ch, s, t] = targets[c, g(ch), s%16, s//16, t]  (same for all ch in group)
    tr = tgt.transpose(0, 1, 3, 2, 4)  # (chunk, group, u, ch_local, t)
    tr = tr.reshape(NCH, 8, 1, NI * 2)
    tr = np.broadcast_to(tr, (NCH, 8, 16, NI * 2)).reshape(NCH, P, 2 * NI)
    return idx, tr.astype(ml_dtypes.bfloat16)


def _run(pred, labels, targets, trace=False):
    labels = np.asarray(labels)
    targets = np.asarray(targets)
    assert pred.shape == (N, C, 2), pred.shape
    assert labels.shape == (N,), labels.shape
    assert targets.shape == (N, 2), targets.shape
    pred_bf = np.asarray(pred, dtype=np.float32).astype(ml_dtypes.bfloat16)

    nc = _build()
    mask = _mask_const()
    in_maps = []
    for c in range(NCORES):
        sl = slice(c * NLOC, (c + 1) * NLOC)
        idx, trep = _host_inputs(labels[sl], targets[sl])
        in_maps.append(
            {
                "pred": pred_bf[sl].reshape(NLOC * C * 2),
                "idx": idx,
                "trep": trep,
                "mask": mask,
            }
        )
    res = run_bass_kernel_spmd(nc, in_maps, list(range(NCORES)), trace=trace)
    total = 0.0
    for r in res.results:
        p = r["partial"].astype(np.float64).reshape(P, NCH, 3)
        # sum(smooth_l1) = 0.5*(sum(d^2) - sum(r1^2) - sum(m2^2))
        total += 0.5 * (p[:, :, 0].sum() - p[:, :, 1].sum() - p[:, :, 2].sum())
    loss = np.float32(total / N)  # = mean * 2 over 2N elements
    return loss, res


def kernel(pred, labels, targets):
    loss, _ = _run(pred, labels, targets)
    return np.asarray(loss, dtype=np.float32)
